# revision 15
# baseline (speedup 1.0000x reference)
"""CosineWeights kernel for Trainium2 (Bass/Tile), SPMD over 8 NeuronCores.

Math (per batch i, head h, memory row j):
    mask2   = mask*mask                                  [H,K]
    proj    = sum_k (mask2*keys)[h,k] * mem[j,k]         [H,J]
    msq     = sum_k kn2*mask2[h,k] * mem[j,k]^2          [H,J]
    kn2     = sum_k (mask*keys)^2                        [H]
    sharp   = softplus(str)[h] * proj / sqrt(kn2*msq)    (EPS folded away; norm ~40 >> 1e-6)
    out     = softmax_j(sharp)

Sharding: data-parallel over batch dim (32 batches -> 8 cores x 4), no
cross-core communication.

Pipeline (v2 -- built from the 88us baseline's trace):
  - All 16 mega-tile loads (f32->bf16 SWDGE cast DMA) are issued up front;
    nat pool holds the core's whole memory slice in SBUF (64KB/partition),
    so the DMA stream never stalls on buffer reuse (~325 GB/s sustained).
    First/last mega-tiles are split into 4 sub-loads to shorten the
    pipeline head/tail.
  - Per mega-tile: PE transposes 128x128 blocks into PSUM bf16; DVE copies
    PSUM->SBUF (memT); ACT squares PSUM->SBUF (memT2). GpSimd does nothing
    but emit DMA descriptors (its 3.7us/tile squares were the baseline's
    tail bottleneck).
  - Matmuls: stationary [128,32] zero-padded tiles (content pre-scaled by
    softplus(str) resp. kn2, at col offset 8*tp), accumulated over
    mega-tiles tp=0..3 into a dense [128,512] PSUM tile per batch:
    partition p = 32q + 8tp + h holds j-chunk t=4tp+q; the 4 q-matmuls of
    a mega-tile share one stationary and run on 4 distinct PE column
    groups (tile_position), so they overlap on the 16 32x32 sub-arrays.
  - Moving operand uses a j-ordered (strided) AP so PSUM free index == j:
    the epilogue needs no strided permute and the out-DMA stays 2KB/run.
  - Epilogue per batch on dense [128,512] tiles: s = exp(-0.5*ln(msq'));
    sharp = proj'*s; exp with fused row-sum; cross-partition fold/
    broadcast via tiny onehot matmuls; no softmax max-subtraction
    (|sharp| <= ~6 -> exp safe in fp32).
  - A few junk matmuls while waiting for the first tile keep the PE HAM
    clock-gate warming so real work starts at 2.4 GHz.
"""

import os

import numpy as np

B, H, J, K = 32, 8, 8192, 128
N_CORES = 8
B_LOC = B // N_CORES  # 4

MEGA = 2048            # j elements per mega-tile
NBLK = MEGA // 128     # 128x128 transpose blocks per mega-tile
NT = J // MEGA         # mega-tiles per batch
T_PER_I = J // 512     # 16 (512-)tiles per batch -> packed 32q+8tp+h on 128 partitions

N_WARM = 4             # junk matmuls to pre-warm the PE clock gate
SPLIT_TILES = ()       # mega-tiles loaded/processed as 4 sub-tiles
DBG_TAPS = None        # debug: {name: dram AP} written during batch-0 epilogue

_NC = None
LAST_RESULTS = None
LAST_EXEC_TIME_NS = None


def _kernel_body(ctx, tc, out_d, mem_d, keys_d, str_d, mask_d):
    import concourse.bass as bass
    from concourse import masks, mybir

    nc = tc.nc
    f32 = mybir.dt.float32
    bf16 = mybir.dt.bfloat16
    AF = mybir.ActivationFunctionType

    const_pool = ctx.enter_context(tc.tile_pool(name="const", bufs=1))
    prep_pool = ctx.enter_context(tc.tile_pool(name="prep", bufs=1))
    nat_pool = ctx.enter_context(tc.tile_pool(name="nat", bufs=B_LOC * NT))
    memT_pool = ctx.enter_context(tc.tile_pool(name="memT", bufs=3))
    memT2_pool = ctx.enter_context(tc.tile_pool(name="memT2", bufs=3))
    epi_pool = ctx.enter_context(tc.tile_pool(name="epi", bufs=2))
    small_pool = ctx.enter_context(tc.tile_pool(name="small", bufs=2))
    psumT_pool = ctx.enter_context(
        tc.tile_pool(name="psumT", bufs=2, space=bass.MemorySpace.PSUM)
    )
    proj_pool = ctx.enter_context(
        tc.tile_pool(name="projps", bufs=2, space=bass.MemorySpace.PSUM)
    )
    msq_pool = ctx.enter_context(
        tc.tile_pool(name="msqps", bufs=1, space=bass.MemorySpace.PSUM)
    )
    tiny_pool = ctx.enter_context(
        tc.tile_pool(name="tinyps", bufs=1, space=bass.MemorySpace.PSUM)
    )

    M_TILES = B_LOC * NT
    nats = {}

    def issue_load(m):
        # partition p holds NBLK consecutive j-rows -> one contiguous 8KB
        # DRAM run per partition (peak DMA efficiency; a j%128 layout that
        # would keep the matmul moving operand j-ordered costs 16x the
        # descriptors and stretches the stream 43->55us).
        i, tp = divmod(m, NT)
        nat = nat_pool.tile([128, MEGA], bf16, tag="nat", name=f"nat{m}")
        src = mem_d[i, tp * MEGA : (tp + 1) * MEGA, :].rearrange(
            "(p c) k -> p c k", p=128
        )
        if m in SPLIT_TILES:
            # 4 sub-loads so downstream transposes can start/finish on the
            # first/last 512-j quarter instead of the whole tile.
            for s in range(4):
                nc.gpsimd.dma_start(
                    nat[:, 512 * s : 512 * (s + 1)].rearrange(
                        "p (c k) -> p c k", c=NBLK // 4
                    ),
                    src[:, 4 * s : 4 * (s + 1), :],
                )
        else:
            nc.gpsimd.dma_start(
                nat[:].rearrange("p (c k) -> p c k", c=NBLK), src
            )
        nats[m] = nat

    # ---- static constants: embedded in the NEFF, DMA'd via HWDGE (sync) ----
    # so gpsimd does nothing but emit the big-load descriptors.
    import ml_dtypes

    IH = B_LOC * H  # 32
    np_bf16 = ml_dtypes.bfloat16

    id_np = np.eye(128, dtype=np_bf16)
    # R[(i,h), 128*i2 + 40*o + e] = (i2==i)*(e==h): one PE transpose of
    # a_sb/b_sb against R materializes the whole zero-padded stationary
    # bank layout in a single [128,512] psum tile.
    lhsR_np = np.zeros((IH, 512), dtype=np_bf16)
    for i in range(B_LOC):
        for o in range(4):
            for h in range(H):
                lhsR_np[8 * i + h, 128 * i + 40 * o + h] = 1.0
    oneT_np = np.tile(np.eye(8, dtype=np.float32), (1, 16))  # [8, 128]
    onehot_np = oneT_np.T.copy()                             # [128, 8]

    id_d = nc.inline_tensor(id_np, name="c_id").ap()
    lhsR_d = nc.inline_tensor(lhsR_np, name="c_lhsR").ap()
    oneT_d = nc.inline_tensor(oneT_np, name="c_oneT").ap()
    onehot_d = nc.inline_tensor(onehot_np, name="c_onehot").ap()

    identity_bf = const_pool.tile([128, 128], bf16)
    nc.sync.dma_start(identity_bf[:], id_d)
    lhsR = const_pool.tile([IH, 512], bf16)
    nc.sync.dma_start(lhsR[:], lhsR_d)
    oneT = const_pool.tile([H, 128], f32)    # oneT[h, 8r+h'] = (h==h')
    nc.sync.dma_start(oneT[:], oneT_d)
    onehot = const_pool.tile([128, H], f32)  # onehot[p, h] = (p%8==h)
    nc.sync.dma_start(onehot[:], onehot_d)

    for m0 in range(M_TILES):
        issue_load(m0)

    warm_rhs = const_pool.tile([128, 512], bf16)
    nc.vector.memset(warm_rhs[:], 0.0)

    # ---- PE warm-up: junk matmuls while the first mega-tile streams in -----
    warm_ps = tiny_pool.tile([128, 512], f32, tag="tiny")
    for _ in range(N_WARM):
        nc.tensor.matmul(
            warm_ps[0:32, :], identity_bf[:, 0:32], warm_rhs[:],
            start=True, stop=True,
        )

    # ---- prep: per-(i,h) scalars and stationary matrices --------------------
    keys_sb = prep_pool.tile([IH, K], f32)
    nc.sync.dma_start(keys_sb[:], keys_d.rearrange("i h k -> (i h) k"))
    mask_sb = prep_pool.tile([IH, K], f32)
    nc.sync.dma_start(mask_sb[:], mask_d.rearrange("i h k -> (i h) k"))
    str_sb = prep_pool.tile([IH, 1], f32)
    nc.sync.dma_start(str_sb[:], str_d.rearrange("i h one -> (i h) one"))

    mask2 = prep_pool.tile([IH, K], f32)
    nc.vector.tensor_mul(mask2[:], mask_sb[:], mask_sb[:])
    a_t = prep_pool.tile([IH, K], f32)
    nc.vector.tensor_mul(a_t[:], mask2[:], keys_sb[:])
    ak = prep_pool.tile([IH, K], f32)
    nc.vector.tensor_mul(ak[:], a_t[:], keys_sb[:])
    kn2 = prep_pool.tile([IH, 1], f32)
    nc.vector.reduce_sum(kn2[:], ak[:], axis=mybir.AxisListType.X)
    # softplus(x) = ln(1 + e^x); no Softplus ACT table on this build.
    # strengths ~ N(0,1) so e^x is comfortably in fp32 range.
    es = prep_pool.tile([IH, 1], f32)
    nc.scalar.activation(es[:], str_sb[:], AF.Exp)
    sp = prep_pool.tile([IH, 1], f32)
    nc.scalar.activation(sp[:], es[:], AF.Ln, bias=1.0)

    a_s = prep_pool.tile([IH, K], f32)  # softplus(str) * mask^2 * keys
    nc.vector.tensor_scalar_mul(a_s[:], a_t[:], sp[:])
    b_s = prep_pool.tile([IH, K], f32)  # kn2 * mask^2
    nc.vector.tensor_scalar_mul(b_s[:], mask2[:], kn2[:])

    a_sb = prep_pool.tile([IH, K], bf16)
    nc.vector.tensor_copy(a_sb[:], a_s[:])
    b_sb = prep_pool.tile([IH, K], bf16)
    nc.vector.tensor_copy(b_sb[:], b_s[:])

    # zero-padded stationary banks: for (i, o) the [128,32] slice at cols
    # v*32 (v = i*4+o) holds a'_i (resp b'_i) at col offset 8*o, zeros
    # elsewhere -- materialized as a_sb.T @ lhsR in one matmul each.
    lhsA = const_pool.tile([128, B_LOC * 4 * 32], bf16)
    lhsB = const_pool.tile([128, B_LOC * 4 * 32], bf16)
    nw = B_LOC * 4 * 32
    lhsA_ps = tiny_pool.tile([128, 512], f32, tag="tiny")
    nc.tensor.matmul(lhsA_ps[:], a_sb[:], lhsR[:], start=True, stop=True)
    nc.vector.tensor_copy(lhsA[:], lhsA_ps[:, 0:nw])
    lhsB_ps = tiny_pool.tile([128, 512], f32, tag="tiny")
    nc.tensor.matmul(lhsB_ps[:], b_sb[:], lhsR[:], start=True, stop=True)
    nc.scalar.copy(lhsB[:], lhsB_ps[:, 0:nw])

    # ---- main loop ----------------------------------------------------------
    for m in range(M_TILES):
        i, tp = divmod(m, NT)
        nat = nats.pop(m)

        psumT = psumT_pool.tile([128, MEGA], bf16, tag="psumT")
        for b in range(NBLK):
            nc.tensor.transpose(
                psumT[:, b * 128 : (b + 1) * 128],
                nat[:, b * 128 : (b + 1) * 128],
                identity_bf[:],
            )

        memT = memT_pool.tile([128, MEGA], bf16, tag="memT")
        memT2 = memT2_pool.tile([128, MEGA], bf16, tag="memT2")
        if m in SPLIT_TILES:
            for s in range(4):
                sl = slice(512 * s, 512 * (s + 1))
                nc.vector.tensor_copy(memT[:, sl], psumT[:, sl])
                nc.scalar.square(memT2[:, sl], psumT[:, sl])
        else:
            nc.vector.tensor_copy(memT[:], psumT[:])
            nc.scalar.square(memT2[:], psumT[:])

        if tp == 0:
            proj_ps = proj_pool.tile([128, 512], f32, tag="proj")
            msq_ps = msq_pool.tile([128, 512], f32, tag="msq")

        # psumT col 128*b + p holds j = NBLK*p + b: chunk q's moving slice
        # is p in [32q, 32q+32) across all b -- dense 32-col inner runs at
        # full PE rate. PSUM free f = 32b + pp <-> j-offset 16pp + b (the
        # epilogue's final scale unscrambles this).
        memT_v = memT[:].rearrange("kk (b pp) -> kk b pp", b=NBLK)
        memT2_v = memT2[:].rearrange("kk (b pp) -> kk b pp", b=NBLK)
        v = i * 4 + tp
        for q in range(4):
            # the 4 q-matmuls share one stationary (content at col offset
            # 8*tp) and target 4 distinct PE column groups -> they overlap
            # on the sub-arrays. PSUM partition 32q+8tp+h = j-chunk 4tp+q.
            nc.tensor.matmul(
                proj_ps[32 * q : 32 * q + 32, :],
                lhsA[:, v * 32 : (v + 1) * 32],
                memT_v[:, :, 32 * q : 32 * q + 32],
                start=(tp == 0),
                stop=(tp == 3),
                tile_position=(0, 32 * q),
            )
        for q in range(4):
            nc.tensor.matmul(
                msq_ps[32 * q : 32 * q + 32, :],
                lhsB[:, v * 32 : (v + 1) * 32],
                memT2_v[:, :, 32 * q : 32 * q + 32],
                start=(tp == 0),
                stop=(tp == 3),
                tile_position=(0, 32 * q),
            )

        if tp == NT - 1:
            # ---- epilogue for batch i on dense [128,512] tiles -------------
            def tap(name, t):
                if DBG_TAPS is not None and i == 0 and name in DBG_TAPS:
                    nc.sync.dma_start(DBG_TAPS[name], t[:])

            lnm = epi_pool.tile([128, 512], f32, tag="lnm")
            nc.scalar.activation(lnm[:], msq_ps[:], AF.Ln)
            tap("lnm", lnm)
            s_t = epi_pool.tile([128, 512], f32, tag="s_t")
            nc.scalar.activation(s_t[:], lnm[:], AF.Exp, scale=-0.5)
            tap("s_t", s_t)
            sharp = epi_pool.tile([128, 512], f32, tag="sharp")
            nc.vector.tensor_mul(sharp[:], proj_ps[:], s_t[:])
            tap("sharp", sharp)
            et = epi_pool.tile([128, 512], f32, tag="et")
            sums = small_pool.tile([128, 1], f32, tag="sums")
            nc.scalar.activation(et[:], sharp[:], AF.Exp, accum_out=sums[:])
            tap("et", et)
            tap("sums", sums)

            # per-h sums across the 16 t-groups: onehot^T @ sums
            hsum_ps = tiny_pool.tile([H, 1], f32, tag="tiny")
            nc.tensor.matmul(
                hsum_ps[:], onehot[:], sums[:], start=True, stop=True
            )
            r8 = small_pool.tile([H, 1], f32, tag="r8")
            nc.vector.reciprocal(r8[:], hsum_ps[:])
            tap("r8", r8)
            # broadcast back to all 128 partitions: oneT^T @ r8
            rb_ps = tiny_pool.tile([128, 1], f32, tag="tiny")
            nc.tensor.matmul(rb_ps[:], oneT[:], r8[:], start=True, stop=True)
            rb = small_pool.tile([128, 1], f32, tag="rb")
            nc.vector.tensor_copy(rb[:], rb_ps[:])
            tap("rb", rb)

            # et free index f = 32*b + pp corresponds to j-offset 16*pp + b
            # within the chunk's 512-j run; permute while applying the
            # softmax scale so the out-DMA writes dense 2KB runs.
            out_t = epi_pool.tile([128, 512], f32, tag="out_t")
            nc.vector.tensor_scalar_mul(
                out_t[:].rearrange("r (pp b) -> r b pp", b=NBLK),
                et[:].rearrange("r (b pp) -> r b pp", b=NBLK),
                rb[:],
            )
            tap("out_t", out_t)
            # partition p = 32q + 8tp + h holds j-chunk t = 4tp + q; the
            # AP balancer caps at 3 dims, so one DMA per q-group.
            out_v = out_d[i].rearrange("h (tp q f) -> q tp h f", tp=4, q=4)
            for q in range(4):
                # SBUF side stays a plain [32, 512] slice: a split partition
                # dim on the SBUF side of a DMA silently mis-addresses.
                nc.sync.dma_start(out_v[q], out_t[32 * q : 32 * q + 32])


def _patch_act_tables():
    """The ACT table-load inserter maps each activation to the first set
    containing it; by default Exp lands in exp_and_others and Ln in
    natural_log, forcing a ~1.5us table switch per Ln<->Exp transition
    (2 per batch epilogue). Reorder so the combined
    natural_log_exp_and_others set is found first -- table loads resolve
    by name, so reordering is safe."""
    import concourse.bacc as bacc

    if getattr(bacc, "_cosine_act_tables_patched", False):
        return
    orig = bacc.get_activation_tables

    def patched(arch):
        from concourse import mybir as _mb

        tables = dict(orig(arch))
        if "natural_log_exp_and_others" not in tables:
            return tables
        # Keep dict order/indices identical (act_func_set_id indexes the
        # act_info.json order); just stop Exp/Ln resolving to the
        # single-function sets so both land in the combined set.
        drop = {_mb.ActivationFunctionType.Exp, _mb.ActivationFunctionType.Ln}
        for name in list(tables):
            if name == "natural_log_exp_and_others":
                continue
            fns = tables[name]
            if isinstance(fns, (set, frozenset)) and (fns & drop):
                tables[name] = fns - drop
        return tables

    bacc.get_activation_tables = patched
    bacc._cosine_act_tables_patched = True


def _build():
    from contextlib import ExitStack

    import concourse.bacc as bacc
    import concourse.tile as tile
    from concourse import mybir

    _patch_act_tables()

    nc = bacc.Bacc(
        "TRN2",
        target_bir_lowering=False,
        debug=False,
        num_devices=N_CORES,
        num_swdge_queues=2,
    )
    f32 = mybir.dt.float32
    mem_d = nc.dram_tensor("memory", [B_LOC, J, K], f32, kind="ExternalInput").ap()
    keys_d = nc.dram_tensor("keys", [B_LOC, H, K], f32, kind="ExternalInput").ap()
    str_d = nc.dram_tensor(
        "strengths", [B_LOC, H, 1], f32, kind="ExternalInput"
    ).ap()
    mask_d = nc.dram_tensor("mask", [B_LOC, H, K], f32, kind="ExternalInput").ap()
    out_d = nc.dram_tensor("out", [B_LOC, H, J], f32, kind="ExternalOutput").ap()

    with tile.TileContext(nc) as tc:
        with ExitStack() as ctx:
            _kernel_body(ctx, tc, out_d, mem_d, keys_d, str_d, mask_d)

    nc.compile()
    return nc


def get_nc():
    global _NC
    if _NC is None:
        _NC = _build()
    return _NC


def kernel(memory, keys, strengths, mask):
    global LAST_RESULTS, LAST_EXEC_TIME_NS
    from concourse.bass_utils import run_bass_kernel_spmd

    nc = get_nc()
    in_maps = []
    for c in range(N_CORES):
        sl = slice(c * B_LOC, (c + 1) * B_LOC)
        in_maps.append(
            {
                "memory": np.ascontiguousarray(memory[sl], dtype=np.float32),
                "keys": np.ascontiguousarray(keys[sl], dtype=np.float32),
                "strengths": np.ascontiguousarray(strengths[sl], dtype=np.float32),
                "mask": np.ascontiguousarray(mask[sl], dtype=np.float32),
            }
        )
    res = run_bass_kernel_spmd(nc, in_maps, list(range(N_CORES)))
    LAST_RESULTS = res
    LAST_EXEC_TIME_NS = res.exec_time_ns
    out = np.concatenate([res.results[c]["out"] for c in range(N_CORES)], axis=0)
    return out.astype(np.float32, copy=False)
